# revision 16
# baseline (speedup 1.0000x reference)
"""Trainium2 Bass kernel for nn_ColumnarTransformerBlock (MoE routing).

B=4 samples, top-2 of 8 experts -> exactly 8 (sample, expert) dispatch
entries, one per NeuronCore. Router + combine run on host (tiny); each core
runs one full transformer-block forward with its expert's weights.

Device kernel layout:
  - activations kept transposed [feature, seq] for projections so weights act
    as lhsT in natural [in, out] layout; v kept natural [seq, feat] with a
    ones column per head (softmax denominators fall out of the ctx matmul);
  - per head-pair: project q,k -> RoPE fused into PSUM eviction -> per head:
    scoresT tiles, exp on ScalarE (no max subtraction; scores are O(1)),
    ctxT accumulation; recip row broadcast via K=1 ones matmul;
  - float32r (tf32-like, full PE rate) for all GEMMs, fp32 elsewhere.
"""

import sys
import types
import numpy as np

import concourse.bass as bass  # noqa: F401
import concourse.tile as tile
from concourse import bacc, mybir
from concourse.bass_utils import run_bass_kernel_spmd
from contextlib import ExitStack

B, SEQ, D = 4, 1024, 1024
H = 16
HD = 64
S_COLS, TOPK = 8, 2
INTER = 768
EPS = 1e-5

F32 = mybir.dt.float32
F32R = mybir.dt.float32r

P = 128
NS = SEQ // 512          # 2 seq chunks of 512
KT = D // P              # 8 contraction chunks for D
KTI = INTER // P         # 6 contraction chunks for INTER
ST = SEQ // P            # 8 seq tiles of 128

_PROGRAM = None


def _rmsnorm(nc, pools, x_sl, out_sl, epsb):
    """out = x * rsqrt(mean(x^2) + EPS) for a [P, D] natural-layout tile.

    x_sl/out_sl are [P, D] APs. Uses two [P, 512] squares to stay in the
    shared 512-wide scratch tag.
    """
    scratch, small = pools
    ssq = small.tile([P, 2], F32, tag="ssq")
    for half in range(2):
        sq = scratch.tile([P, 512], F32, tag="t512")
        nc.vector.tensor_mul(sq[:], x_sl[:, half * 512:(half + 1) * 512],
                             x_sl[:, half * 512:(half + 1) * 512])
        nc.vector.reduce_sum(out=ssq[:, half:half + 1], in_=sq[:],
                             axis=mybir.AxisListType.X)
    tot = small.tile([P, 1], F32, tag="ssqt")
    nc.vector.tensor_add(tot[:], ssq[:, 0:1], ssq[:, 1:2])
    st_ = small.tile([P, 1], F32, tag="st")
    nc.scalar.activation(st_[:], tot[:], mybir.ActivationFunctionType.Sqrt,
                         bias=epsb[:], scale=1.0 / D)
    rstd = small.tile([P, 1], F32, tag="rstd")
    nc.vector.reciprocal(rstd[:], st_[:])
    nc.vector.tensor_scalar_mul(out_sl, x_sl, rstd[:])


def _build_program():
    nc = bacc.Bacc("TRN2", target_bir_lowering=False, debug=False)

    d_xnat = nc.dram_tensor("x_nat", [SEQ, D], F32, kind="ExternalInput").ap()
    d_xt = nc.dram_tensor("x_t", [D, SEQ], F32R, kind="ExternalInput").ap()
    d_wqk = nc.dram_tensor("wqk", [D, 2 * D], F32R, kind="ExternalInput").ap()
    d_wv = nc.dram_tensor("wv", [D, D], F32R, kind="ExternalInput").ap()
    d_wo = nc.dram_tensor("wo", [D, D], F32R, kind="ExternalInput").ap()
    d_wgu = nc.dram_tensor("wgu", [D, 2 * INTER], F32R, kind="ExternalInput").ap()
    d_wd = nc.dram_tensor("wd", [INTER, D], F32R, kind="ExternalInput").ap()
    d_cos = nc.dram_tensor("cos_rep", [P, SEQ], F32, kind="ExternalInput").ap()
    d_sin = nc.dram_tensor("sin_rep", [P, SEQ], F32, kind="ExternalInput").ap()
    d_nsin = nc.dram_tensor("nsin_rep", [P, SEQ], F32, kind="ExternalInput").ap()
    d_ident = nc.dram_tensor("ident", [P, P], F32R, kind="ExternalInput").ap()
    d_ones = nc.dram_tensor("ones64", [1, 64], F32R, kind="ExternalInput").ap()
    d_y = nc.dram_tensor("y", [SEQ, D], F32, kind="ExternalOutput").ap()

    xt_r = d_xt.rearrange("(kc kp) s -> kp kc s", kp=P)
    wqk_r = d_wqk.rearrange("(kc kp) c -> kp kc c", kp=P)
    wv_r = d_wv.rearrange("(kc kp) c -> kp kc c", kp=P)
    wo_r = d_wo.rearrange("(kc kp) c -> kp kc c", kp=P)
    wgu_r = d_wgu.rearrange("(kc kp) c -> kp kc c", kp=P)
    wd_r = d_wd.rearrange("(kc kp) c -> kp kc c", kp=P)
    xnat_r = d_xnat.rearrange("(t p) d -> p t d", p=P)
    y_r = d_y.rearrange("(t p) d -> p t d", p=P)

    with tile.TileContext(nc) as tc, ExitStack() as ctx:
        big = ctx.enter_context(tc.tile_pool(name="big", bufs=1))
        qkpool = ctx.enter_context(tc.tile_pool(name="qkpool", bufs=3))
        consts = ctx.enter_context(tc.tile_pool(name="consts", bufs=1))
        wstream = ctx.enter_context(tc.tile_pool(name="wstream", bufs=2))
        mstream = ctx.enter_context(tc.tile_pool(name="mstream", bufs=3))
        scratch = ctx.enter_context(tc.tile_pool(name="scratch", bufs=4))
        x1s = ctx.enter_context(tc.tile_pool(name="x1s", bufs=2))
        small = ctx.enter_context(tc.tile_pool(name="small", bufs=4))
        expool = ctx.enter_context(tc.tile_pool(name="expool", bufs=3))
        ps_mm = ctx.enter_context(tc.tile_pool(name="ps_mm", bufs=3, space="PSUM"))
        ps_ctx = ctx.enter_context(tc.tile_pool(name="ps_ctx", bufs=4, space="PSUM"))
        ps_sm = ctx.enter_context(tc.tile_pool(name="ps_sm", bufs=1, space="PSUM"))

        # ---- constants ----
        cos_rep = consts.tile([P, SEQ], F32, tag="cos")
        sin_rep = consts.tile([P, SEQ], F32, tag="sin")
        nsin_rep = consts.tile([P, SEQ], F32, tag="nsin")
        ident = consts.tile([P, P], F32R, tag="ident")
        ones64 = consts.tile([1, 64], F32R, tag="ones")
        epsb = consts.tile([P, 1], F32, tag="epsb")
        nc.vector.memset(epsb[:], EPS)
        nc.sync.dma_start(cos_rep[:], d_cos[:])
        nc.sync.dma_start(sin_rep[:], d_sin[:])
        nc.sync.dma_start(nsin_rep[:], d_nsin[:])
        nc.sync.dma_start(ident[:], d_ident[:])
        nc.sync.dma_start(ones64[:], d_ones[:])

        # ---- persistent tensors; tags pair sequential live ranges --------
        xt_sb = big.tile([P, KT, SEQ], F32R, tag="slot_xt")      # proj phases
        v_ext = big.tile([P, ST, 16 * 65], F32R, tag="slot_v")   # B -> attn
        ctxT = big.tile([P, KT, SEQ], F32R, tag="slot_ctx")      # attn -> D

        nc.sync.dma_start(xt_sb[:], xt_r[:])

        # ================= Phase B: v projection (natural) ===============
        vx = v_ext.rearrange("p t (h f) -> p t h f", f=65)
        nc.vector.memset(v_ext.bitcast(F32)[:], 1.0)   # ones cols survive evictions
        for n in range(NS):
            wvt = wstream.tile([P, KT, 512], F32R, tag="w2m")
            nc.sync.dma_start(wvt[:], wv_r[:, :, n * 512:(n + 1) * 512])
            for t in range(ST):
                psum = ps_mm.tile([P, 512], F32, tag="mm")
                for kc in range(KT):
                    nc.tensor.matmul(
                        psum[:], xt_sb[:, kc, t * P:(t + 1) * P], wvt[:, kc, :],
                        start=(kc == 0), stop=(kc == KT - 1))
                nc.vector.tensor_copy(
                    vx[:, t, n * 8:(n + 1) * 8, 0:64],
                    psum.rearrange("p (h f) -> p h f", f=64))

        # ========== Phase A+C: per head-pair qk proj + RoPE + attention ==
        for hp in range(H // 2):
            qk_t = []
            for which in range(2):               # 0 = q tile, 1 = k tile
                m = hp if which == 0 else KT + hp
                wtile = mstream.tile([P, KT, P], F32R, tag="m512")
                nc.sync.dma_start(wtile[:], wqk_r[:, :, m * P:(m + 1) * P])
                rtile = qkpool.tile([P, SEQ], F32R, tag="qk")
                qk_t.append(rtile)
                for n in range(NS):
                    csl = slice(n * 512, (n + 1) * 512)
                    psum = ps_mm.tile([P, 512], F32, tag="mm")
                    for kc in range(KT):
                        nc.tensor.matmul(
                            psum[:], wtile[:, kc, :], xt_sb[:, kc, csl],
                            start=(kc == 0), stop=(kc == KT - 1))
                    ev = scratch.tile([P, 512], F32, tag="t512")
                    nc.scalar.copy(ev[:], psum[:])   # PSUM->SBUF on ScalarE
                    rot = scratch.tile([P, 512], F32, tag="t512")
                    cp = scratch.tile([P, 512], F32, tag="t512")
                    # all-SBUF ops run DVE's 2x mode; 2 muls go to GpSimd
                    nc.gpsimd.tensor_mul(rot[0:32], ev[32:64],
                                         nsin_rep[32:64, csl])
                    nc.vector.tensor_mul(rot[32:64], ev[0:32],
                                         sin_rep[0:32, csl])
                    nc.gpsimd.tensor_mul(rot[64:96], ev[96:128],
                                         nsin_rep[96:128, csl])
                    nc.vector.tensor_mul(rot[96:128], ev[64:96],
                                         sin_rep[64:96, csl])
                    nc.vector.tensor_mul(cp[:], ev[:], cos_rep[:, csl])
                    nc.vector.tensor_add(rtile[:, csl], cp[:], rot[:])
            qt, kt = qk_t
            for hh in range(2):                  # heads 2*hp + hh
                h = 2 * hp + hh
                qoff = hh * 64
                ctx_ps = [ps_ctx.tile([65, 512], F32, tag="ctx", name=f"ctxps{n}")
                          for n in range(NS)]
                for s2t in range(ST):
                    for n in range(NS):
                        sc_ps = ps_mm.tile([P, 512], F32, tag="mm")
                        nc.tensor.matmul(
                            sc_ps[:],
                            kt[qoff:qoff + 64, s2t * P:(s2t + 1) * P],
                            qt[qoff:qoff + 64, n * 512:(n + 1) * 512],
                            start=True, stop=True)
                        ex = expool.tile([P, 512], F32R, tag="exp")
                        nc.scalar.activation(
                            ex[:], sc_ps[:], mybir.ActivationFunctionType.Exp,
                            scale=0.125)
                        nc.tensor.matmul(
                            ctx_ps[n][:], v_ext[:, s2t, 65 * h:65 * h + 65],
                            ex[:], start=(s2t == 0), stop=(s2t == ST - 1))
                for n in range(NS):
                    recip = small.tile([1, 512], F32R, tag="recip", bufs=2)
                    with nc.allow_low_precision(reason="f32r rounding of softmax recip"):
                        nc.vector.reciprocal(recip[:], ctx_ps[n][64:65, :])
                    bc_ps = ps_sm.tile([64, 512], F32, tag="sm")
                    nc.tensor.matmul(bc_ps[:], ones64[:], recip[:],
                                     start=True, stop=True)
                    ct_sb = small.tile([64, 512], F32, tag="cts", bufs=2)
                    nc.scalar.copy(ct_sb[:], ctx_ps[n][0:64, :])
                    bc_sb = small.tile([64, 512], F32, tag="bcs", bufs=2)
                    nc.scalar.copy(bc_sb[:], bc_ps[:])
                    nc.vector.tensor_mul(
                        ctxT[qoff:qoff + 64, hp, n * 512:(n + 1) * 512],
                        ct_sb[:], bc_sb[:])

        # ================= Phase D: Wo + residual + rmsnorm ==============
        x1n = big.tile([P, ST, D], F32R, tag="slot_xt")   # reuses xt slot
        wot = [wstream.tile([P, KT, 512], F32R, tag="w2m", name=f"wot{n}")
               for n in range(NS)]
        for n in range(NS):
            nc.sync.dma_start(wot[n][:], wo_r[:, :, n * 512:(n + 1) * 512])
        for t in range(ST):
            xnt = mstream.tile([P, D], F32, tag="m512")
            nc.sync.dma_start(xnt[:], xnat_r[:, t, :])
            x1 = x1s.tile([P, D], F32, tag="x1")
            for n in range(NS):
                psum = ps_mm.tile([P, 512], F32, tag="mm")
                for kc in range(KT):
                    nc.tensor.matmul(
                        psum[:], ctxT[:, kc, t * P:(t + 1) * P], wot[n][:, kc, :],
                        start=(kc == 0), stop=(kc == KT - 1))
                nc.vector.tensor_add(
                    x1[:, n * 512:(n + 1) * 512], xnt[:, n * 512:(n + 1) * 512],
                    psum[:])
            _rmsnorm(nc, (scratch, small), x1[:], x1n[:, t, :], epsb)

        # ================= Phase E: transpose x1n -> x1nT ================
        x1nT = big.tile([P, KT, SEQ], F32R, tag="slot_ctx")  # after ctxT dies
        for t in range(ST):
            for fcc in range(KT):
                tp = ps_sm.tile([P, P], F32R, tag="sm")
                nc.tensor.transpose(
                    tp[:], x1n[:, t, fcc * P:(fcc + 1) * P], ident[:])
                nc.vector.tensor_copy(x1nT[:, fcc, t * P:(t + 1) * P], tp[:])

        # ================= Phase F: Wgu + silu-gate ======================
        hT = big.tile([P, KTI, SEQ], F32R, tag="slot_v")  # reuses v slot
        for g in range(KTI):
            wg = mstream.tile([P, KT, P], F32R, tag="m512")
            wu = mstream.tile([P, KT, P], F32R, tag="m512")
            nc.sync.dma_start(wg[:], wgu_r[:, :, g * P:(g + 1) * P])
            nc.sync.dma_start(wu[:], wgu_r[:, :, INTER + g * P:INTER + (g + 1) * P])
            for n in range(NS):
                csl = slice(n * 512, (n + 1) * 512)
                gps = ps_mm.tile([P, 512], F32, tag="mm")
                ups = ps_mm.tile([P, 512], F32, tag="mm")
                for kc in range(KT):
                    nc.tensor.matmul(
                        gps[:], wg[:, kc, :], x1nT[:, kc, csl],
                        start=(kc == 0), stop=(kc == KT - 1))
                for kc in range(KT):
                    nc.tensor.matmul(
                        ups[:], wu[:, kc, :], x1nT[:, kc, csl],
                        start=(kc == 0), stop=(kc == KT - 1))
                sil = scratch.tile([P, 512], F32, tag="t512")
                nc.scalar.activation(
                    sil[:], gps[:], mybir.ActivationFunctionType.Silu)
                nc.vector.tensor_mul(hT[:, g, csl], sil[:], ups[:])

        # ================= Phase G: Wd + residual + rmsnorm + out ========
        x2c = big.tile([P, ST, D], F32, tag="slot_ctx")   # reuses ctx slot
        for n in range(NS):
            wdt = wstream.tile([P, KTI, 512], F32R, tag="w2m")
            nc.sync.dma_start(wdt[:], wd_r[:, :, n * 512:(n + 1) * 512])
            for t in range(ST):
                psum = ps_mm.tile([P, 512], F32, tag="mm")
                for kc in range(KTI):
                    nc.tensor.matmul(
                        psum[:], hT[:, kc, t * P:(t + 1) * P], wdt[:, kc, :],
                        start=(kc == 0), stop=(kc == KTI - 1))
                nc.vector.tensor_add(
                    x2c[:, t, n * 512:(n + 1) * 512],
                    x1n[:, t, n * 512:(n + 1) * 512], psum[:])
        for t in range(ST):
            yt = x1s.tile([P, D], F32, tag="x1")
            _rmsnorm(nc, (scratch, small), x2c[:, t, :], yt[:], epsb)
            nc.sync.dma_start(y_r[:, t, :], yt[:])

    nc.compile()
    return nc


def _get_program():
    global _PROGRAM
    if _PROGRAM is None:
        _PROGRAM = _build_program()
    return _PROGRAM


def _routing(hidden_states, Wr, temperature):
    temp = np.clip(temperature.astype(np.float32), 0.1, 10.0)
    mean = hidden_states.astype(np.float64).mean(axis=1)
    logits = (mean @ Wr.astype(np.float64).T) / np.float64(temp[0])
    logits = logits.astype(np.float32)
    order = np.argsort(-logits, axis=-1, kind="stable")[:, :TOPK]
    topv = np.take_along_axis(logits, order, axis=-1).astype(np.float32)
    m = topv.max(axis=-1, keepdims=True)
    e = np.exp(topv - m)
    w = (e / e.sum(axis=-1, keepdims=True)).astype(np.float32)
    return logits, order, w


def _install_trace_shim():
    import contextlib
    import ctypes

    lib = ctypes.CDLL("/opt/axon/libaxon_pjrt.so")
    if not hasattr(lib, "axon_start_nrt_profile"):
        return
    lib.axon_start_nrt_profile.argtypes = [
        ctypes.POINTER(ctypes.c_int64), ctypes.c_size_t]
    lib.axon_start_nrt_profile.restype = ctypes.c_int64
    lib.axon_stop_nrt_profile.argtypes = [ctypes.c_char_p]
    lib.axon_stop_nrt_profile.restype = ctypes.c_int64

    @contextlib.contextmanager
    def _hook(output_dir, device_ids):
        import jax
        jax.devices()
        if device_ids:
            ids = (ctypes.c_int64 * len(device_ids))(*device_ids)
            rc = lib.axon_start_nrt_profile(ids, len(device_ids))
        else:
            rc = lib.axon_start_nrt_profile(None, 0)
        if rc != 0:
            raise RuntimeError(f"axon_start_nrt_profile rc={rc}")
        try:
            yield
        finally:
            n = lib.axon_stop_nrt_profile(str(output_dir).encode())
            if n < 0:
                raise RuntimeError(f"axon_stop_nrt_profile rc={n}")

    mod = types.ModuleType("antenv.axon_hooks")
    mod.get_axon_ntff_profile_hook = lambda: _hook
    mod.set_axon_ntff_profile_hook = lambda h: None
    sys.modules["antenv.axon_hooks"] = mod


def kernel(hidden_states, cos, sin, Wr, temperature, Wqkv, Wo, Wgu, Wd,
           _trace=False):
    hidden_states = np.asarray(hidden_states, np.float32)
    cos = np.asarray(cos, np.float32)
    sin = np.asarray(sin, np.float32)
    Wr = np.asarray(Wr, np.float32)
    temperature = np.asarray(temperature, np.float32)
    Wqkv = np.asarray(Wqkv, np.float32)
    Wo = np.asarray(Wo, np.float32)
    Wgu = np.asarray(Wgu, np.float32)
    Wd = np.asarray(Wd, np.float32)

    logits, topk_idx, weights = _routing(hidden_states, Wr, temperature)

    cos32 = np.ascontiguousarray(cos[:, :32].T)          # [32, SEQ]
    sin32 = np.ascontiguousarray(sin[:, :32].T)
    cos_rep = np.tile(cos32, (4, 1)).astype(np.float32)
    sin_rep = np.tile(sin32, (4, 1)).astype(np.float32)
    nsin_rep = (-sin_rep).astype(np.float32)
    ident = np.eye(P, dtype=np.float32)
    ones64 = np.ones((1, 64), np.float32)

    in_maps = []
    for i in range(B * TOPK):
        b, e = i // TOPK, int(topk_idx[i // TOPK, i % TOPK])
        x = np.ascontiguousarray(hidden_states[b])
        in_maps.append({
            "x_nat": x,
            "x_t": np.ascontiguousarray(x.T),
            "wqk": np.ascontiguousarray(Wqkv[e][:, :2 * D]),
            "wv": np.ascontiguousarray(Wqkv[e][:, 2 * D:]),
            "wo": np.ascontiguousarray(Wo[e]),
            "wgu": np.ascontiguousarray(Wgu[e]),
            "wd": np.ascontiguousarray(Wd[e]),
            "cos_rep": cos_rep, "sin_rep": sin_rep, "nsin_rep": nsin_rep,
            "ident": ident, "ones64": ones64,
        })

    nc = _get_program()
    if _trace:
        _install_trace_shim()
    res = run_bass_kernel_spmd(nc, in_maps, list(range(B * TOPK)), trace=_trace)

    result = np.zeros((B, SEQ, D), np.float32)
    for i in range(B * TOPK):
        b = i // TOPK
        w = np.float32(weights[b, i % TOPK])
        result[b] += w * res.results[i]["y"]

    if _trace:
        kernel.last_exec_time_ns = res.exec_time_ns
        kernel.last_results = res
    return result, logits


# revision 21
# speedup vs baseline: 1.1789x; 1.1789x over previous
"""Trainium2 Bass kernel for nn_ColumnarTransformerBlock (MoE routing).

B=4 samples, top-2 of 8 experts -> exactly 8 (sample, expert) dispatch
entries, one per NeuronCore. Router + combine run on host (tiny); each core
runs one full transformer-block forward with its expert's weights.

Device kernel layout:
  - activations kept transposed [feature, seq] for projections so weights act
    as lhsT in natural [in, out] layout; v kept natural [seq, feat] with a
    ones column per head (softmax denominators fall out of the ctx matmul);
  - per head-pair: project q,k -> RoPE fused into PSUM eviction -> per head:
    scoresT tiles, exp on ScalarE (no max subtraction; scores are O(1)),
    ctxT accumulation; recip row broadcast via K=1 ones matmul;
  - float32r (tf32-like, full PE rate) for all GEMMs, fp32 elsewhere.
"""

import sys
import types
import numpy as np

import concourse.bass as bass  # noqa: F401
import concourse.tile as tile
from concourse import bacc, mybir
from concourse.bass_utils import run_bass_kernel_spmd
from contextlib import ExitStack

B, SEQ, D = 4, 1024, 1024
H = 16
HD = 64
S_COLS, TOPK = 8, 2
INTER = 768
EPS = 1e-5

F32 = mybir.dt.float32
F32R = mybir.dt.float32r
BF16 = mybir.dt.bfloat16

P = 128
NS = SEQ // 512          # 2 seq chunks of 512
KT = D // P              # 8 contraction chunks for D
KTI = INTER // P         # 6 contraction chunks for INTER
ST = SEQ // P            # 8 seq tiles of 128

_PROGRAM = None


def _rmsnorm(nc, pools, x_sl, out_sl, epsb):
    """out = x * rsqrt(mean(x^2) + EPS) for a [P, D] natural-layout tile.

    x_sl/out_sl are [P, D] APs. Uses two [P, 512] squares to stay in the
    shared 512-wide scratch tag.
    """
    scratch, small = pools
    ssq = small.tile([P, 2], F32, tag="ssq")
    for half in range(2):
        sq = scratch.tile([P, 512], F32, tag="t512")
        nc.vector.tensor_mul(sq[:], x_sl[:, half * 512:(half + 1) * 512],
                             x_sl[:, half * 512:(half + 1) * 512])
        nc.vector.reduce_sum(out=ssq[:, half:half + 1], in_=sq[:],
                             axis=mybir.AxisListType.X)
    tot = small.tile([P, 1], F32, tag="ssqt")
    nc.vector.tensor_add(tot[:], ssq[:, 0:1], ssq[:, 1:2])
    st_ = small.tile([P, 1], F32, tag="st")
    nc.scalar.activation(st_[:], tot[:], mybir.ActivationFunctionType.Sqrt,
                         bias=epsb[:], scale=1.0 / D)
    rstd = small.tile([P, 1], F32, tag="rstd")
    nc.vector.reciprocal(rstd[:], st_[:])
    nc.vector.tensor_scalar_mul(out_sl, x_sl, rstd[:])


def _build_program():
    nc = bacc.Bacc("TRN2", target_bir_lowering=False, debug=False)

    d_xnat = nc.dram_tensor("x_nat", [SEQ, D], F32, kind="ExternalInput").ap()
    d_xt = nc.dram_tensor("x_t", [D, SEQ], F32R, kind="ExternalInput").ap()
    d_wqk = nc.dram_tensor("wqk", [D, 2 * D], F32R, kind="ExternalInput").ap()
    d_wv = nc.dram_tensor("wv", [D, D], F32R, kind="ExternalInput").ap()
    d_wo = nc.dram_tensor("wo", [D, D], F32R, kind="ExternalInput").ap()
    d_wgu = nc.dram_tensor("wgu", [D, 2 * INTER], F32R, kind="ExternalInput").ap()
    d_wd = nc.dram_tensor("wd", [INTER, D], F32R, kind="ExternalInput").ap()
    d_cos = nc.dram_tensor("cos_rep", [P, SEQ], BF16, kind="ExternalInput").ap()
    d_sin = nc.dram_tensor("sin_rep", [P, SEQ], BF16, kind="ExternalInput").ap()
    d_nsin = nc.dram_tensor("nsin_rep", [P, SEQ], BF16, kind="ExternalInput").ap()
    d_ident = nc.dram_tensor("ident", [P, P], F32R, kind="ExternalInput").ap()
    d_ones = nc.dram_tensor("ones64", [1, 64], F32R, kind="ExternalInput").ap()
    d_y = nc.dram_tensor("y", [SEQ, D], F32, kind="ExternalOutput").ap()

    xt_r = d_xt.rearrange("(kc kp) s -> kp kc s", kp=P)
    wqk_r = d_wqk.rearrange("(kc kp) c -> kp kc c", kp=P)
    wv_r = d_wv.rearrange("(kc kp) c -> kp kc c", kp=P)
    wo_r = d_wo.rearrange("(kc kp) c -> kp kc c", kp=P)
    wgu_r = d_wgu.rearrange("(kc kp) c -> kp kc c", kp=P)
    wd_r = d_wd.rearrange("(kc kp) c -> kp kc c", kp=P)
    xnat_r = d_xnat.rearrange("(t p) d -> p t d", p=P)
    y_r = d_y.rearrange("(t p) d -> p t d", p=P)

    with tile.TileContext(nc) as tc, ExitStack() as ctx:
        big = ctx.enter_context(tc.tile_pool(name="big", bufs=1))
        qkpool = ctx.enter_context(tc.tile_pool(name="qkpool", bufs=3))
        consts = ctx.enter_context(tc.tile_pool(name="consts", bufs=1))
        wstream = ctx.enter_context(tc.tile_pool(name="wstream", bufs=2))
        mstream = ctx.enter_context(tc.tile_pool(name="mstream", bufs=3))
        scratch = ctx.enter_context(tc.tile_pool(name="scratch", bufs=4))
        x1s = ctx.enter_context(tc.tile_pool(name="x1s", bufs=2))
        small = ctx.enter_context(tc.tile_pool(name="small", bufs=4))
        expool = ctx.enter_context(tc.tile_pool(name="expool", bufs=10))
        ps_mm = ctx.enter_context(tc.tile_pool(name="ps_mm", bufs=4, space="PSUM"))
        ps_ctx = ctx.enter_context(tc.tile_pool(name="ps_ctx", bufs=3, space="PSUM"))
        ps_sm = ctx.enter_context(tc.tile_pool(name="ps_sm", bufs=1, space="PSUM"))

        # ---- constants ----
        cos_rep = consts.tile([P, SEQ], BF16, tag="cos")
        sin_rep = consts.tile([P, SEQ], BF16, tag="sin")
        nsin_rep = consts.tile([P, SEQ], BF16, tag="nsin")
        ident = consts.tile([P, P], F32R, tag="ident")
        ones64 = consts.tile([1, 64], F32R, tag="ones")
        epsb = consts.tile([P, 1], F32, tag="epsb")
        nc.vector.memset(epsb[:], EPS)
        nc.sync.dma_start(cos_rep[:], d_cos[:])
        nc.sync.dma_start(sin_rep[:], d_sin[:])
        nc.sync.dma_start(nsin_rep[:], d_nsin[:])
        nc.sync.dma_start(ident[:], d_ident[:])
        nc.sync.dma_start(ones64[:], d_ones[:])

        # ---- persistent tensors; tags pair sequential live ranges --------
        xt_sb = big.tile([P, KT, SEQ], F32R, tag="slot_xt")      # proj phases
        v_ext = big.tile([P, ST, 16 * 65], BF16, tag="slot_v")   # B -> attn
        ctxT = big.tile([P, KT, SEQ], F32R, tag="slot_ctx")      # attn -> D

        nc.sync.dma_start(xt_sb[:], xt_r[:])

        # ================= Phase B: v projection (natural) ===============
        vx = v_ext.rearrange("p t (h f) -> p t h f", f=65)
        nc.vector.memset(v_ext[:], 1.0)   # ones cols survive evictions
        for n in range(NS):
            wvt = wstream.tile([P, KT, 512], F32R, tag="w2m")
            nc.sync.dma_start(wvt[:], wv_r[:, :, n * 512:(n + 1) * 512])
            for t in range(ST):
                psum = ps_mm.tile([P, 512], F32, tag="mm")
                for kc in range(KT):
                    nc.tensor.matmul(
                        psum[:], xt_sb[:, kc, t * P:(t + 1) * P], wvt[:, kc, :],
                        start=(kc == 0), stop=(kc == KT - 1))
                nc.vector.tensor_copy(
                    vx[:, t, n * 8:(n + 1) * 8, 0:64],
                    psum.rearrange("p (h f) -> p h f", f=64))

        # ========== Phase A+C: per head-pair qk proj + RoPE + attention ==
        # Attention interior runs in bf16: matmuls at 1 cyc/row with fast
        # weight load, RoPE on DVE's 2x bf16 mode, exp tiles at 1 KB/part.
        for hp in range(H // 2):
            qk_t = []
            for which in range(2):               # 0 = q tile, 1 = k tile
                m = hp if which == 0 else KT + hp
                wtile = mstream.tile([P, KT, P], F32R, tag="m512")
                nc.sync.dma_start(wtile[:], wqk_r[:, :, m * P:(m + 1) * P])
                rtile = qkpool.tile([P, SEQ], BF16, tag="qk")
                qk_t.append(rtile)
                for n in range(NS):
                    csl = slice(n * 512, (n + 1) * 512)
                    psum = ps_mm.tile([P, 512], F32, tag="mm")
                    for kc in range(KT):
                        nc.tensor.matmul(
                            psum[:], wtile[:, kc, :], xt_sb[:, kc, csl],
                            start=(kc == 0), stop=(kc == KT - 1))
                    ev = scratch.tile([P, 512], BF16, tag="tb16")
                    nc.scalar.copy(ev[:], psum[:])   # PSUM->SBUF + bf16 cast
                    rot = scratch.tile([P, 512], BF16, tag="tb16")
                    cp = scratch.tile([P, 512], BF16, tag="tb16")
                    # all-bf16 SBUF ops hit DVE's 2x mode
                    nc.vector.tensor_mul(rot[0:32], ev[32:64],
                                         nsin_rep[32:64, csl])
                    nc.vector.tensor_mul(rot[32:64], ev[0:32],
                                         sin_rep[0:32, csl])
                    nc.vector.tensor_mul(rot[64:96], ev[96:128],
                                         nsin_rep[96:128, csl])
                    nc.vector.tensor_mul(rot[96:128], ev[64:96],
                                         sin_rep[64:96, csl])
                    nc.vector.tensor_mul(cp[:], ev[:], cos_rep[:, csl])
                    nc.vector.tensor_add(rtile[:, csl], cp[:], rot[:])
            qt, kt = qk_t
            for hh in range(2):                  # heads 2*hp + hh
                h = 2 * hp + hh
                qoff = hh * 64
                for n in range(NS):
                    # dense batch of 8 scores+exp, then 8 ctx accumulations:
                    # PE never waits on ScalarE at steady state
                    exs = []
                    for s2t in range(ST):
                        sc_ps = ps_mm.tile([P, 512], F32, tag="mm")
                        nc.tensor.matmul(
                            sc_ps[:],
                            kt[qoff:qoff + 64, s2t * P:(s2t + 1) * P],
                            qt[qoff:qoff + 64, n * 512:(n + 1) * 512],
                            start=True, stop=True)
                        ex = expool.tile([P, 512], BF16, tag="exp",
                                         name=f"ex{s2t}")
                        nc.scalar.activation(
                            ex[:], sc_ps[:], mybir.ActivationFunctionType.Exp,
                            scale=0.125)
                        exs.append(ex)
                    ctx_acc = ps_ctx.tile([65, 512], F32, tag="ctx")
                    for s2t in range(ST):
                        nc.tensor.matmul(
                            ctx_acc[:], v_ext[:, s2t, 65 * h:65 * h + 65],
                            exs[s2t][:], start=(s2t == 0), stop=(s2t == ST - 1))
                    recip = small.tile([1, 512], F32R, tag="recip", bufs=2)
                    with nc.allow_low_precision(reason="f32r softmax recip"):
                        nc.vector.reciprocal(recip[:], ctx_acc[64:65, :])
                    bc_ps = ps_sm.tile([64, 512], F32, tag="sm")
                    nc.tensor.matmul(bc_ps[:], ones64[:], recip[:],
                                     start=True, stop=True)
                    ct_sb = small.tile([64, 512], F32, tag="cts", bufs=2)
                    nc.scalar.copy(ct_sb[:], ctx_acc[0:64, :])
                    bc_sb = small.tile([64, 512], F32, tag="bcs", bufs=2)
                    nc.scalar.copy(bc_sb[:], bc_ps[:])
                    nc.vector.tensor_mul(
                        ctxT[qoff:qoff + 64, hp, n * 512:(n + 1) * 512],
                        ct_sb[:], bc_sb[:])

        # ================= Phase D: Wo + residual + rmsnorm ==============
        x1n = big.tile([P, ST, D], F32R, tag="slot_xt")   # reuses xt slot
        wot = [wstream.tile([P, KT, 512], F32R, tag="w2m", name=f"wot{n}")
               for n in range(NS)]
        for n in range(NS):
            nc.sync.dma_start(wot[n][:], wo_r[:, :, n * 512:(n + 1) * 512])
        for t in range(ST):
            xnt = mstream.tile([P, D], F32, tag="m512")
            nc.sync.dma_start(xnt[:], xnat_r[:, t, :])
            x1 = x1s.tile([P, D], F32, tag="x1")
            for n in range(NS):
                psum = ps_mm.tile([P, 512], F32, tag="mm")
                for kc in range(KT):
                    nc.tensor.matmul(
                        psum[:], ctxT[:, kc, t * P:(t + 1) * P], wot[n][:, kc, :],
                        start=(kc == 0), stop=(kc == KT - 1))
                nc.vector.tensor_add(
                    x1[:, n * 512:(n + 1) * 512], xnt[:, n * 512:(n + 1) * 512],
                    psum[:])
            _rmsnorm(nc, (scratch, small), x1[:], x1n[:, t, :], epsb)

        # ================= Phase E: transpose x1n -> x1nT ================
        x1nT = big.tile([P, KT, SEQ], F32R, tag="slot_ctx")  # after ctxT dies
        for t in range(ST):
            for fcc in range(KT):
                tp = ps_sm.tile([P, P], F32R, tag="sm")
                nc.tensor.transpose(
                    tp[:], x1n[:, t, fcc * P:(fcc + 1) * P], ident[:])
                nc.vector.tensor_copy(x1nT[:, fcc, t * P:(t + 1) * P], tp[:])

        # ================= Phase F: Wgu + silu-gate ======================
        hT = big.tile([P, KTI, SEQ], F32R, tag="slot_v")  # reuses v slot
        for g in range(KTI):
            wg = mstream.tile([P, KT, P], F32R, tag="m512")
            wu = mstream.tile([P, KT, P], F32R, tag="m512")
            nc.sync.dma_start(wg[:], wgu_r[:, :, g * P:(g + 1) * P])
            nc.sync.dma_start(wu[:], wgu_r[:, :, INTER + g * P:INTER + (g + 1) * P])
            for n in range(NS):
                csl = slice(n * 512, (n + 1) * 512)
                gps = ps_mm.tile([P, 512], F32, tag="mm")
                ups = ps_mm.tile([P, 512], F32, tag="mm")
                for kc in range(KT):
                    nc.tensor.matmul(
                        gps[:], wg[:, kc, :], x1nT[:, kc, csl],
                        start=(kc == 0), stop=(kc == KT - 1))
                for kc in range(KT):
                    nc.tensor.matmul(
                        ups[:], wu[:, kc, :], x1nT[:, kc, csl],
                        start=(kc == 0), stop=(kc == KT - 1))
                sil = scratch.tile([P, 512], F32, tag="t512")
                nc.scalar.activation(
                    sil[:], gps[:], mybir.ActivationFunctionType.Silu)
                nc.vector.tensor_mul(hT[:, g, csl], sil[:], ups[:])

        # ================= Phase G: Wd + residual + rmsnorm + out ========
        x2c = big.tile([P, ST, D], F32, tag="slot_ctx")   # reuses ctx slot
        for n in range(NS):
            wdt = wstream.tile([P, KTI, 512], F32R, tag="w2m")
            nc.sync.dma_start(wdt[:], wd_r[:, :, n * 512:(n + 1) * 512])
            for t in range(ST):
                psum = ps_mm.tile([P, 512], F32, tag="mm")
                for kc in range(KTI):
                    nc.tensor.matmul(
                        psum[:], hT[:, kc, t * P:(t + 1) * P], wdt[:, kc, :],
                        start=(kc == 0), stop=(kc == KTI - 1))
                nc.vector.tensor_add(
                    x2c[:, t, n * 512:(n + 1) * 512],
                    x1n[:, t, n * 512:(n + 1) * 512], psum[:])
        for t in range(ST):
            yt = x1s.tile([P, D], F32, tag="x1")
            _rmsnorm(nc, (scratch, small), x2c[:, t, :], yt[:], epsb)
            nc.sync.dma_start(y_r[:, t, :], yt[:])

    nc.compile()
    return nc


def _get_program():
    global _PROGRAM
    if _PROGRAM is None:
        _PROGRAM = _build_program()
    return _PROGRAM


def _routing(hidden_states, Wr, temperature):
    temp = np.clip(temperature.astype(np.float32), 0.1, 10.0)
    mean = hidden_states.astype(np.float64).mean(axis=1)
    logits = (mean @ Wr.astype(np.float64).T) / np.float64(temp[0])
    logits = logits.astype(np.float32)
    order = np.argsort(-logits, axis=-1, kind="stable")[:, :TOPK]
    topv = np.take_along_axis(logits, order, axis=-1).astype(np.float32)
    m = topv.max(axis=-1, keepdims=True)
    e = np.exp(topv - m)
    w = (e / e.sum(axis=-1, keepdims=True)).astype(np.float32)
    return logits, order, w


def _install_trace_shim():
    import contextlib
    import ctypes

    lib = ctypes.CDLL("/opt/axon/libaxon_pjrt.so")
    if not hasattr(lib, "axon_start_nrt_profile"):
        return
    lib.axon_start_nrt_profile.argtypes = [
        ctypes.POINTER(ctypes.c_int64), ctypes.c_size_t]
    lib.axon_start_nrt_profile.restype = ctypes.c_int64
    lib.axon_stop_nrt_profile.argtypes = [ctypes.c_char_p]
    lib.axon_stop_nrt_profile.restype = ctypes.c_int64

    @contextlib.contextmanager
    def _hook(output_dir, device_ids):
        import jax
        jax.devices()
        if device_ids:
            ids = (ctypes.c_int64 * len(device_ids))(*device_ids)
            rc = lib.axon_start_nrt_profile(ids, len(device_ids))
        else:
            rc = lib.axon_start_nrt_profile(None, 0)
        if rc != 0:
            raise RuntimeError(f"axon_start_nrt_profile rc={rc}")
        try:
            yield
        finally:
            n = lib.axon_stop_nrt_profile(str(output_dir).encode())
            if n < 0:
                raise RuntimeError(f"axon_stop_nrt_profile rc={n}")

    mod = types.ModuleType("antenv.axon_hooks")
    mod.get_axon_ntff_profile_hook = lambda: _hook
    mod.set_axon_ntff_profile_hook = lambda h: None
    sys.modules["antenv.axon_hooks"] = mod


def kernel(hidden_states, cos, sin, Wr, temperature, Wqkv, Wo, Wgu, Wd,
           _trace=False):
    hidden_states = np.asarray(hidden_states, np.float32)
    cos = np.asarray(cos, np.float32)
    sin = np.asarray(sin, np.float32)
    Wr = np.asarray(Wr, np.float32)
    temperature = np.asarray(temperature, np.float32)
    Wqkv = np.asarray(Wqkv, np.float32)
    Wo = np.asarray(Wo, np.float32)
    Wgu = np.asarray(Wgu, np.float32)
    Wd = np.asarray(Wd, np.float32)

    logits, topk_idx, weights = _routing(hidden_states, Wr, temperature)

    cos32 = np.ascontiguousarray(cos[:, :32].T)          # [32, SEQ]
    sin32 = np.ascontiguousarray(sin[:, :32].T)
    import ml_dtypes
    cos_rep = np.tile(cos32, (4, 1)).astype(ml_dtypes.bfloat16)
    sin_rep = np.tile(sin32, (4, 1)).astype(ml_dtypes.bfloat16)
    nsin_rep = (-sin_rep).astype(ml_dtypes.bfloat16)
    ident = np.eye(P, dtype=np.float32)
    ones64 = np.ones((1, 64), np.float32)

    in_maps = []
    for i in range(B * TOPK):
        b, e = i // TOPK, int(topk_idx[i // TOPK, i % TOPK])
        x = np.ascontiguousarray(hidden_states[b])
        in_maps.append({
            "x_nat": x,
            "x_t": np.ascontiguousarray(x.T),
            "wqk": np.ascontiguousarray(Wqkv[e][:, :2 * D]),
            "wv": np.ascontiguousarray(Wqkv[e][:, 2 * D:]),
            "wo": np.ascontiguousarray(Wo[e]),
            "wgu": np.ascontiguousarray(Wgu[e]),
            "wd": np.ascontiguousarray(Wd[e]),
            "cos_rep": cos_rep, "sin_rep": sin_rep, "nsin_rep": nsin_rep,
            "ident": ident, "ones64": ones64,
        })

    nc = _get_program()
    if _trace:
        _install_trace_shim()
    res = run_bass_kernel_spmd(nc, in_maps, list(range(B * TOPK)), trace=_trace)

    result = np.zeros((B, SEQ, D), np.float32)
    for i in range(B * TOPK):
        b = i // TOPK
        w = np.float32(weights[b, i % TOPK])
        result[b] += w * res.results[i]["y"]

    if _trace:
        kernel.last_exec_time_ns = res.exec_time_ns
        kernel.last_results = res
    return result, logits


# revision 22
# speedup vs baseline: 1.2899x; 1.0941x over previous
"""Trainium2 Bass kernel for nn_ColumnarTransformerBlock (MoE routing).

B=4 samples, top-2 of 8 experts -> exactly 8 (sample, expert) dispatch
entries, one per NeuronCore. Router + combine run on host (tiny); each core
runs one full transformer-block forward with its expert's weights.

Device kernel layout:
  - activations kept transposed [feature, seq] for projections so weights act
    as lhsT in natural [in, out] layout; v kept natural [seq, feat] with a
    ones column per head (softmax denominators fall out of the ctx matmul);
  - per head-pair: project q,k -> RoPE fused into PSUM eviction -> per head:
    scoresT tiles, exp on ScalarE (no max subtraction; scores are O(1)),
    ctxT accumulation; recip row broadcast via K=1 ones matmul;
  - float32r (tf32-like, full PE rate) for all GEMMs, fp32 elsewhere.
"""

import sys
import types
import numpy as np

import concourse.bass as bass  # noqa: F401
import concourse.tile as tile
from concourse import bacc, mybir
from concourse.bass_utils import run_bass_kernel_spmd
from contextlib import ExitStack

B, SEQ, D = 4, 1024, 1024
H = 16
HD = 64
S_COLS, TOPK = 8, 2
INTER = 768
EPS = 1e-5

F32 = mybir.dt.float32
F32R = mybir.dt.float32r
BF16 = mybir.dt.bfloat16

P = 128
NS = SEQ // 512          # 2 seq chunks of 512
KT = D // P              # 8 contraction chunks for D
KTI = INTER // P         # 6 contraction chunks for INTER
ST = SEQ // P            # 8 seq tiles of 128

_PROGRAM = None


def _rmsnorm(nc, pools, x_sl, out_sl, epsb):
    """out = x * rsqrt(mean(x^2) + EPS) for a [P, D] natural-layout tile.

    x_sl/out_sl are [P, D] APs. Uses two [P, 512] squares to stay in the
    shared 512-wide scratch tag.
    """
    scratch, small = pools
    ssq = small.tile([P, 2], F32, tag="ssq")
    for half in range(2):
        sq = scratch.tile([P, 512], F32, tag="t512")
        nc.vector.tensor_mul(sq[:], x_sl[:, half * 512:(half + 1) * 512],
                             x_sl[:, half * 512:(half + 1) * 512])
        nc.vector.reduce_sum(out=ssq[:, half:half + 1], in_=sq[:],
                             axis=mybir.AxisListType.X)
    tot = small.tile([P, 1], F32, tag="ssqt")
    nc.vector.tensor_add(tot[:], ssq[:, 0:1], ssq[:, 1:2])
    st_ = small.tile([P, 1], F32, tag="st")
    nc.scalar.activation(st_[:], tot[:], mybir.ActivationFunctionType.Sqrt,
                         bias=epsb[:], scale=1.0 / D)
    rstd = small.tile([P, 1], F32, tag="rstd")
    nc.vector.reciprocal(rstd[:], st_[:])
    nc.vector.tensor_scalar_mul(out_sl, x_sl, rstd[:])


def _build_program():
    nc = bacc.Bacc("TRN2", target_bir_lowering=False, debug=False)

    d_xnat = nc.dram_tensor("x_nat", [SEQ, D], F32, kind="ExternalInput").ap()
    d_xt = nc.dram_tensor("x_t", [D, SEQ], F32R, kind="ExternalInput").ap()
    d_wqk = nc.dram_tensor("wqk", [D, 2 * D], F32R, kind="ExternalInput").ap()
    d_wv = nc.dram_tensor("wv", [D, D], F32R, kind="ExternalInput").ap()
    d_wo = nc.dram_tensor("wo", [D, D], F32R, kind="ExternalInput").ap()
    d_wgu = nc.dram_tensor("wgu", [D, 2 * INTER], F32R, kind="ExternalInput").ap()
    d_wd = nc.dram_tensor("wd", [INTER, D], F32R, kind="ExternalInput").ap()
    d_cos = nc.dram_tensor("cos_rep", [P, SEQ], BF16, kind="ExternalInput").ap()
    d_sin = nc.dram_tensor("sin_rep", [P, SEQ], BF16, kind="ExternalInput").ap()
    d_nsin = nc.dram_tensor("nsin_rep", [P, SEQ], BF16, kind="ExternalInput").ap()
    d_ident = nc.dram_tensor("ident", [P, P], F32R, kind="ExternalInput").ap()
    d_ones = nc.dram_tensor("ones64", [1, 64], F32R, kind="ExternalInput").ap()
    d_y = nc.dram_tensor("y", [SEQ, D], F32, kind="ExternalOutput").ap()

    xt_r = d_xt.rearrange("(kc kp) s -> kp kc s", kp=P)
    wqk_r = d_wqk.rearrange("(kc kp) c -> kp kc c", kp=P)
    wv_r = d_wv.rearrange("(kc kp) c -> kp kc c", kp=P)
    wo_r = d_wo.rearrange("(kc kp) c -> kp kc c", kp=P)
    wgu_r = d_wgu.rearrange("(kc kp) c -> kp kc c", kp=P)
    wd_r = d_wd.rearrange("(kc kp) c -> kp kc c", kp=P)
    xnat_r = d_xnat.rearrange("(t p) d -> p t d", p=P)
    y_r = d_y.rearrange("(t p) d -> p t d", p=P)

    with tile.TileContext(nc) as tc, ExitStack() as ctx:
        big = ctx.enter_context(tc.tile_pool(name="big", bufs=1))
        qkpool = ctx.enter_context(tc.tile_pool(name="qkpool", bufs=3))
        consts = ctx.enter_context(tc.tile_pool(name="consts", bufs=1))
        wstream = ctx.enter_context(tc.tile_pool(name="wstream", bufs=2))
        mstream = ctx.enter_context(tc.tile_pool(name="mstream", bufs=3))
        scratch = ctx.enter_context(tc.tile_pool(name="scratch", bufs=4))
        x1s = ctx.enter_context(tc.tile_pool(name="x1s", bufs=2))
        small = ctx.enter_context(tc.tile_pool(name="small", bufs=4))
        expool = ctx.enter_context(tc.tile_pool(name="expool", bufs=10))
        ps_mm = ctx.enter_context(tc.tile_pool(name="ps_mm", bufs=4, space="PSUM"))
        ps_ctx = ctx.enter_context(tc.tile_pool(name="ps_ctx", bufs=3, space="PSUM"))
        ps_sm = ctx.enter_context(tc.tile_pool(name="ps_sm", bufs=1, space="PSUM"))
        drampool = ctx.enter_context(tc.tile_pool(name="drampool", bufs=4, space="DRAM"))

        # ---- constants ----
        cos_rep = consts.tile([P, SEQ], BF16, tag="cos")
        sin_rep = consts.tile([P, SEQ], BF16, tag="sin")
        nsin_rep = consts.tile([P, SEQ], BF16, tag="nsin")
        ident = consts.tile([P, P], F32R, tag="ident")
        ones64 = consts.tile([1, 64], F32R, tag="ones")
        epsb = consts.tile([P, 1], F32, tag="epsb")
        nc.vector.memset(epsb[:], EPS)
        nc.sync.dma_start(cos_rep[:], d_cos[:])
        nc.sync.dma_start(sin_rep[:], d_sin[:])
        nc.sync.dma_start(nsin_rep[:], d_nsin[:])
        nc.sync.dma_start(ident[:], d_ident[:])
        nc.sync.dma_start(ones64[:], d_ones[:])

        # ---- persistent tensors; tags pair sequential live ranges --------
        xt_sb = big.tile([P, KT, SEQ], F32R, tag="slot_xt")      # proj phases
        v_ext = big.tile([P, ST, 16 * 65], BF16, tag="slot_v")   # B -> attn
        ctxT = big.tile([P, KT, SEQ], F32R, tag="slot_ctx")      # attn -> D

        nc.sync.dma_start(xt_sb[:], xt_r[:])

        # ================= Phase B: v projection (natural) ===============
        vx = v_ext.rearrange("p t (h f) -> p t h f", f=65)
        nc.vector.memset(v_ext[:], 1.0)   # ones cols survive evictions
        for n in range(NS):
            wvt = wstream.tile([P, KT, 512], F32R, tag="w2m")
            nc.sync.dma_start(wvt[:], wv_r[:, :, n * 512:(n + 1) * 512])
            for t in range(ST):
                psum = ps_mm.tile([P, 512], F32, tag="mm")
                for kc in range(KT):
                    nc.tensor.matmul(
                        psum[:], xt_sb[:, kc, t * P:(t + 1) * P], wvt[:, kc, :],
                        start=(kc == 0), stop=(kc == KT - 1))
                nc.vector.tensor_copy(
                    vx[:, t, n * 8:(n + 1) * 8, 0:64],
                    psum.rearrange("p (h f) -> p h f", f=64))

        # ========== Phase A+C: per head-pair qk proj + RoPE + attention ==
        # Attention interior runs in bf16: matmuls at 1 cyc/row with fast
        # weight load, RoPE on DVE's 2x bf16 mode, exp tiles at 1 KB/part.
        for hp in range(H // 2):
            qk_t = []
            for which in range(2):               # 0 = q tile, 1 = k tile
                m = hp if which == 0 else KT + hp
                wtile = mstream.tile([P, KT, P], F32R, tag="m512")
                nc.sync.dma_start(wtile[:], wqk_r[:, :, m * P:(m + 1) * P])
                rtile = qkpool.tile([P, SEQ], BF16, tag="qk")
                qk_t.append(rtile)
                for n in range(NS):
                    csl = slice(n * 512, (n + 1) * 512)
                    psum = ps_mm.tile([P, 512], F32, tag="mm")
                    for kc in range(KT):
                        nc.tensor.matmul(
                            psum[:], wtile[:, kc, :], xt_sb[:, kc, csl],
                            start=(kc == 0), stop=(kc == KT - 1))
                    ev = scratch.tile([P, 512], BF16, tag="tb16")
                    nc.scalar.copy(ev[:], psum[:])   # PSUM->SBUF + bf16 cast
                    rot = scratch.tile([P, 512], BF16, tag="tb16")
                    cp = scratch.tile([P, 512], BF16, tag="tb16")
                    # all-bf16 SBUF ops hit DVE's 2x mode
                    nc.vector.tensor_mul(rot[0:32], ev[32:64],
                                         nsin_rep[32:64, csl])
                    nc.vector.tensor_mul(rot[32:64], ev[0:32],
                                         sin_rep[0:32, csl])
                    nc.vector.tensor_mul(rot[64:96], ev[96:128],
                                         nsin_rep[96:128, csl])
                    nc.vector.tensor_mul(rot[96:128], ev[64:96],
                                         sin_rep[64:96, csl])
                    nc.vector.tensor_mul(cp[:], ev[:], cos_rep[:, csl])
                    nc.vector.tensor_add(rtile[:, csl], cp[:], rot[:])
            qt, kt = qk_t
            for hh in range(2):                  # heads 2*hp + hh
                h = 2 * hp + hh
                qoff = hh * 64
                for n in range(NS):
                    # dense batch of 8 scores+exp, then 8 ctx accumulations:
                    # PE never waits on ScalarE at steady state
                    exs = []
                    for s2t in range(ST):
                        sc_ps = ps_mm.tile([P, 512], F32, tag="mm")
                        nc.tensor.matmul(
                            sc_ps[:],
                            kt[qoff:qoff + 64, s2t * P:(s2t + 1) * P],
                            qt[qoff:qoff + 64, n * 512:(n + 1) * 512],
                            start=True, stop=True)
                        ex = expool.tile([P, 512], BF16, tag="exp",
                                         name=f"ex{s2t}")
                        nc.scalar.activation(
                            ex[:], sc_ps[:], mybir.ActivationFunctionType.Exp,
                            scale=0.125)
                        exs.append(ex)
                    ctx_acc = ps_ctx.tile([65, 512], F32, tag="ctx")
                    for s2t in range(ST):
                        nc.tensor.matmul(
                            ctx_acc[:], v_ext[:, s2t, 65 * h:65 * h + 65],
                            exs[s2t][:], start=(s2t == 0), stop=(s2t == ST - 1))
                    recip = small.tile([1, 512], F32, tag="recip", bufs=2)
                    nc.vector.reciprocal(recip[:], ctx_acc[64:65, :])
                    rscr = drampool.tile([1, 512], F32, tag="rscr", bufs=4)
                    nc.sync.dma_start(rscr[:], recip[:])
                    bc_sb = small.tile([64, 512], F32, tag="bcs", bufs=2)
                    nc.sync.dma_start(bc_sb[:], rscr.to_broadcast([64, 512]))
                    ct_sb = small.tile([64, 512], F32, tag="cts", bufs=2)
                    nc.vector.tensor_copy(ct_sb[:], ctx_acc[0:64, :])
                    nc.vector.tensor_mul(
                        ctxT[qoff:qoff + 64, hp, n * 512:(n + 1) * 512],
                        ct_sb[:], bc_sb[:])

        # ================= Phase D: Wo + residual + rmsnorm ==============
        x1n = big.tile([P, ST, D], F32R, tag="slot_xt")   # reuses xt slot
        wot = [wstream.tile([P, KT, 512], F32R, tag="w2m", name=f"wot{n}")
               for n in range(NS)]
        for n in range(NS):
            nc.sync.dma_start(wot[n][:], wo_r[:, :, n * 512:(n + 1) * 512])
        for t in range(ST):
            xnt = mstream.tile([P, D], F32, tag="m512")
            nc.sync.dma_start(xnt[:], xnat_r[:, t, :])
            x1 = x1s.tile([P, D], F32, tag="x1")
            for n in range(NS):
                psum = ps_mm.tile([P, 512], F32, tag="mm")
                for kc in range(KT):
                    nc.tensor.matmul(
                        psum[:], ctxT[:, kc, t * P:(t + 1) * P], wot[n][:, kc, :],
                        start=(kc == 0), stop=(kc == KT - 1))
                nc.vector.tensor_add(
                    x1[:, n * 512:(n + 1) * 512], xnt[:, n * 512:(n + 1) * 512],
                    psum[:])
            _rmsnorm(nc, (scratch, small), x1[:], x1n[:, t, :], epsb)

        # ================= Phase E: transpose x1n -> x1nT ================
        x1nT = big.tile([P, KT, SEQ], F32R, tag="slot_ctx")  # after ctxT dies
        for t in range(ST):
            for fcc in range(KT):
                tp = ps_sm.tile([P, P], F32R, tag="sm")
                nc.tensor.transpose(
                    tp[:], x1n[:, t, fcc * P:(fcc + 1) * P], ident[:])
                nc.vector.tensor_copy(x1nT[:, fcc, t * P:(t + 1) * P], tp[:])

        # ================= Phase F: Wgu + silu-gate ======================
        hT = big.tile([P, KTI, SEQ], F32R, tag="slot_v")  # reuses v slot
        for g in range(KTI):
            wg = mstream.tile([P, KT, P], F32R, tag="m512")
            wu = mstream.tile([P, KT, P], F32R, tag="m512")
            nc.sync.dma_start(wg[:], wgu_r[:, :, g * P:(g + 1) * P])
            nc.sync.dma_start(wu[:], wgu_r[:, :, INTER + g * P:INTER + (g + 1) * P])
            for n in range(NS):
                csl = slice(n * 512, (n + 1) * 512)
                gps = ps_mm.tile([P, 512], F32, tag="mm")
                ups = ps_mm.tile([P, 512], F32, tag="mm")
                for kc in range(KT):
                    nc.tensor.matmul(
                        gps[:], wg[:, kc, :], x1nT[:, kc, csl],
                        start=(kc == 0), stop=(kc == KT - 1))
                for kc in range(KT):
                    nc.tensor.matmul(
                        ups[:], wu[:, kc, :], x1nT[:, kc, csl],
                        start=(kc == 0), stop=(kc == KT - 1))
                sil = scratch.tile([P, 512], F32, tag="t512")
                nc.scalar.activation(
                    sil[:], gps[:], mybir.ActivationFunctionType.Silu)
                nc.vector.tensor_mul(hT[:, g, csl], sil[:], ups[:])

        # ================= Phase G: Wd + residual + rmsnorm + out ========
        x2c = big.tile([P, ST, D], F32, tag="slot_ctx")   # reuses ctx slot
        for n in range(NS):
            wdt = wstream.tile([P, KTI, 512], F32R, tag="w2m")
            nc.sync.dma_start(wdt[:], wd_r[:, :, n * 512:(n + 1) * 512])
            for t in range(ST):
                psum = ps_mm.tile([P, 512], F32, tag="mm")
                for kc in range(KTI):
                    nc.tensor.matmul(
                        psum[:], hT[:, kc, t * P:(t + 1) * P], wdt[:, kc, :],
                        start=(kc == 0), stop=(kc == KTI - 1))
                nc.vector.tensor_add(
                    x2c[:, t, n * 512:(n + 1) * 512],
                    x1n[:, t, n * 512:(n + 1) * 512], psum[:])
        for t in range(ST):
            yt = x1s.tile([P, D], F32, tag="x1")
            _rmsnorm(nc, (scratch, small), x2c[:, t, :], yt[:], epsb)
            nc.sync.dma_start(y_r[:, t, :], yt[:])

    nc.compile()
    return nc


def _get_program():
    global _PROGRAM
    if _PROGRAM is None:
        _PROGRAM = _build_program()
    return _PROGRAM


def _routing(hidden_states, Wr, temperature):
    temp = np.clip(temperature.astype(np.float32), 0.1, 10.0)
    mean = hidden_states.astype(np.float64).mean(axis=1)
    logits = (mean @ Wr.astype(np.float64).T) / np.float64(temp[0])
    logits = logits.astype(np.float32)
    order = np.argsort(-logits, axis=-1, kind="stable")[:, :TOPK]
    topv = np.take_along_axis(logits, order, axis=-1).astype(np.float32)
    m = topv.max(axis=-1, keepdims=True)
    e = np.exp(topv - m)
    w = (e / e.sum(axis=-1, keepdims=True)).astype(np.float32)
    return logits, order, w


def _install_trace_shim():
    import contextlib
    import ctypes

    lib = ctypes.CDLL("/opt/axon/libaxon_pjrt.so")
    if not hasattr(lib, "axon_start_nrt_profile"):
        return
    lib.axon_start_nrt_profile.argtypes = [
        ctypes.POINTER(ctypes.c_int64), ctypes.c_size_t]
    lib.axon_start_nrt_profile.restype = ctypes.c_int64
    lib.axon_stop_nrt_profile.argtypes = [ctypes.c_char_p]
    lib.axon_stop_nrt_profile.restype = ctypes.c_int64

    @contextlib.contextmanager
    def _hook(output_dir, device_ids):
        import jax
        jax.devices()
        if device_ids:
            ids = (ctypes.c_int64 * len(device_ids))(*device_ids)
            rc = lib.axon_start_nrt_profile(ids, len(device_ids))
        else:
            rc = lib.axon_start_nrt_profile(None, 0)
        if rc != 0:
            raise RuntimeError(f"axon_start_nrt_profile rc={rc}")
        try:
            yield
        finally:
            n = lib.axon_stop_nrt_profile(str(output_dir).encode())
            if n < 0:
                raise RuntimeError(f"axon_stop_nrt_profile rc={n}")

    mod = types.ModuleType("antenv.axon_hooks")
    mod.get_axon_ntff_profile_hook = lambda: _hook
    mod.set_axon_ntff_profile_hook = lambda h: None
    sys.modules["antenv.axon_hooks"] = mod


def kernel(hidden_states, cos, sin, Wr, temperature, Wqkv, Wo, Wgu, Wd,
           _trace=False):
    hidden_states = np.asarray(hidden_states, np.float32)
    cos = np.asarray(cos, np.float32)
    sin = np.asarray(sin, np.float32)
    Wr = np.asarray(Wr, np.float32)
    temperature = np.asarray(temperature, np.float32)
    Wqkv = np.asarray(Wqkv, np.float32)
    Wo = np.asarray(Wo, np.float32)
    Wgu = np.asarray(Wgu, np.float32)
    Wd = np.asarray(Wd, np.float32)

    logits, topk_idx, weights = _routing(hidden_states, Wr, temperature)

    cos32 = np.ascontiguousarray(cos[:, :32].T)          # [32, SEQ]
    sin32 = np.ascontiguousarray(sin[:, :32].T)
    import ml_dtypes
    cos_rep = np.tile(cos32, (4, 1)).astype(ml_dtypes.bfloat16)
    sin_rep = np.tile(sin32, (4, 1)).astype(ml_dtypes.bfloat16)
    nsin_rep = (-sin_rep).astype(ml_dtypes.bfloat16)
    ident = np.eye(P, dtype=np.float32)
    ones64 = np.ones((1, 64), np.float32)

    in_maps = []
    for i in range(B * TOPK):
        b, e = i // TOPK, int(topk_idx[i // TOPK, i % TOPK])
        x = np.ascontiguousarray(hidden_states[b])
        in_maps.append({
            "x_nat": x,
            "x_t": np.ascontiguousarray(x.T),
            "wqk": np.ascontiguousarray(Wqkv[e][:, :2 * D]),
            "wv": np.ascontiguousarray(Wqkv[e][:, 2 * D:]),
            "wo": np.ascontiguousarray(Wo[e]),
            "wgu": np.ascontiguousarray(Wgu[e]),
            "wd": np.ascontiguousarray(Wd[e]),
            "cos_rep": cos_rep, "sin_rep": sin_rep, "nsin_rep": nsin_rep,
            "ident": ident, "ones64": ones64,
        })

    nc = _get_program()
    if _trace:
        _install_trace_shim()
    res = run_bass_kernel_spmd(nc, in_maps, list(range(B * TOPK)), trace=_trace)

    result = np.zeros((B, SEQ, D), np.float32)
    for i in range(B * TOPK):
        b = i // TOPK
        w = np.float32(weights[b, i % TOPK])
        result[b] += w * res.results[i]["y"]

    if _trace:
        kernel.last_exec_time_ns = res.exec_time_ns
        kernel.last_results = res
    return result, logits
